# revision 2
# baseline (speedup 1.0000x reference)
"""Trainium2 Bass kernel for nn_AttentionBlock (B=16, C=512, H=W=32, 8 heads, d_k=64).

Sharding: data-parallel over batch; each of the 8 NeuronCores computes 2 batches.

ACT-bound design: the softmax exp stream (128 ACTIVATE instructions of
[128,1024] each, ~1.05us apiece) is the pacing engine; everything else is
scheduled to keep it saturated.  The scalar queue carries ONLY exp
instructions once the stream starts; head DMAs ride it before that.

Precision split (absmax-budget driven):
  q/k path fp16   : x, W_qk fp16 matmuls -> scores fp16 (softmax is very
                    sensitive to score noise; fp8 here costs ~2e-2 absmax)
  exp -> float8e5 : ACT writes e5m2 directly (max 57344 holds exp(12.2-4.5);
                    e4m3's 240 overflows -> inf/NaN, and a larger shift
                    flushes 40% of the softmax mass to zero)
  attn@v          : DoubleRow fp8 matmul over token-tile pairs, v stored
                    e4m3 (from an fp16 projection), ones-augmented so rows
                    0:64 of the result hold sumexp
  out projection  : DoubleRow fp8 (res e4m3, W_out e4m3) + fp16 residual
                    xpb = x + b_out + b_v@W_out (host-folded; b_v is never
                    added to v: softmax rows sum to 1)

PSUM budget (8 banks): scores 2 x [128,1024] + attnv res 3 x [128,512] +
filler accumulator 1 x [128,512].  The tail reuses the freed score banks.
"""
from collections import deque

import numpy as np
import ml_dtypes

import concourse.bass as bass
from concourse import bacc
import concourse.mybir as mybir
import concourse.tile as tile
from concourse import bass_utils

F32 = mybir.dt.float32
F16 = mybir.dt.float16
F8 = mybir.dt.float8e4
F8E5 = mybir.dt.float8e5
AF = mybir.ActivationFunctionType
ALU = mybir.AluOpType
PM = mybir.MatmulPerfMode

N_HEADS = 8
DK = 64
SCALE = DK ** -0.5
EXP_SHIFT = -4.5
C = 512
N = 1024            # tokens per batch (32*32)
NB = 2              # batches per core
NCORES = 8
NCH = C // 128      # 4 contraction chunks
NCH2 = NCH // 2     # 2 chunk pairs (DoubleRow)
NT = N // 128       # 8 token tiles
NT2 = NT // 2       # 4 token-tile pairs (DoubleRow)
NPAIR = N_HEADS // 2


def build():
    nc = bacc.Bacc(None, target_bir_lowering=False, num_swdge_queues=4)
    x16_d = nc.dram_tensor("x16", (NB, C, N), F16, kind="ExternalInput")
    xpb_d = nc.dram_tensor("xpb", (NB, C, N), F16, kind="ExternalInput")
    wqk_d = nc.dram_tensor("w_qk", (128, NPAIR, NCH, 2, 128), F16,
                           kind="ExternalInput")
    bqkt_d = nc.dram_tensor("b_qk_t", (128, 2, NPAIR), F32,
                            kind="ExternalInput")
    wv_d = nc.dram_tensor("w_v", (128, NCH, C), F16, kind="ExternalInput")
    wout_d = nc.dram_tensor("w_out", (NCH2, 128, 2, C), F8,
                            kind="ExternalInput")
    zeros_d = nc.dram_tensor("zeros", (1, N), F16, kind="ExternalInput")
    y_d = nc.dram_tensor("y", (NB, C, N), F16, kind="ExternalOutput")

    with tile.TileContext(nc) as tc:
        with (
            tc.tile_pool(name="const", bufs=1) as const,
            tc.tile_pool(name="persist", bufs=1) as persist,
            tc.tile_pool(name="sbwork", bufs=3) as sbwork,
            tc.tile_pool(name="sbexp", bufs=4) as sbexp,
            tc.tile_pool(name="ps_s", bufs=2, space="PSUM") as ps_s,
            tc.tile_pool(name="ps_res", bufs=3, space="PSUM") as ps_res,
            tc.tile_pool(name="ps_acc", bufs=1, space="PSUM") as ps_acc,
        ):
            # ---- persistent tiles ----
            x16 = persist.tile([128, NCH, N], F16, name="x16")
            xpb_r = [persist.tile([128, NCH, N], F16, name=f"xpb{bb}")
                     for bb in range(NB)]
            qpair = [persist.tile([128, N], F16, name=f"qpair{p}")
                     for p in range(NPAIR)]
            kpad = [[persist.tile([128, N], F16, name=f"kpad{p}_{s}")
                     for s in range(2)] for p in range(NPAIR)]
            v8 = persist.tile([128, NT2, 2, N_HEADS, 128], F8, name="v8")
            resc8 = [[persist.tile([128, 2, N], F8, name=f"res{bb}_{c2}")
                      for c2 in range(NCH2)] for bb in range(NB)]
            wqk = const.tile([128, NPAIR, NCH, 2, 128], F16, name="wqk")
            wv = const.tile([128, NCH, C], F16, name="wv")

            # ---- staging DMAs, priority-ordered by first use ----
            # Per-chunk transfers spread over all three DMA-capable queues:
            # any PE-idle window in the head locks the HAM clock gate to
            # ~2.0GHz for the whole run, so the critical pieces (wqk pair 0,
            # x16 nh0, bqkt, kpad0 zeros) must land while the short warmup
            # block runs.  The scalar queue is exp-only once the stream
            # starts; until then it carries the wqk/wv weights.
            xv = x16_d[0].rearrange("(c p) n -> p c n", p=128)
            nc.scalar.dma_start(wqk[:, 0], wqk_d[:, 0])       # pair 0
            nc.sync.dma_start(x16[:, 0:2, 0:512], xv[:, 0:2, 0:512])
            nc.gpsimd.dma_start(x16[:, 2:4, 0:512], xv[:, 2:4, 0:512])
            bqkt = const.tile([128, 2, NPAIR], F32)
            nc.sync.dma_start(bqkt[:], bqkt_d[:])
            # pair-0 zero halves: the very first score matmul reads them
            nc.sync.dma_start(kpad[0][0][64:128, :],
                              zeros_d[:].to_broadcast([64, N]))
            nc.gpsimd.dma_start(kpad[0][1][0:64, :],
                                zeros_d[:].to_broadcast([64, N]))
            nc.scalar.dma_start(wqk[:, 1], wqk_d[:, 1])       # pair 1
            # x16 nh1 halves
            nc.sync.dma_start(x16[:, 0:2, 512:1024], xv[:, 0:2, 512:1024])
            nc.gpsimd.dma_start(x16[:, 2:4, 512:1024], xv[:, 2:4, 512:1024])
            nc.scalar.dma_start(wv[:], wv_d[:])
            nc.scalar.dma_start(wqk[:, 2:NPAIR], wqk_d[:, 2:NPAIR])
            for p in range(1, NPAIR):
                nc.sync.dma_start(kpad[p][0][64:128, :],
                                  zeros_d[:].to_broadcast([64, N]))
                nc.gpsimd.dma_start(kpad[p][1][0:64, :],
                                    zeros_d[:].to_broadcast([64, N]))

            # HAM warmup: dummy matmuls on memset data run during the initial
            # DMA wait so the real matmuls ramp the PE clock.
            warm = const.tile([128, 512], F16)
            nc.vector.memset(warm[:], 0.5)
            warm_ps = ps_acc.tile([128, 512], F32, tag="acc", name="warm_ps")
            for r in range(10):
                nc.tensor.matmul(warm_ps[:], warm[:, 0:128], warm[:],
                                 start=(r == 0), stop=(r == 9))
            # preload the exp table set during the DMA head
            warm_exp = const.tile([128, 8], F16)
            nc.scalar.activation(out=warm_exp[:], in_=warm[:, 0:8],
                                 func=AF.Exp, scale=1.0)
            expbias = const.tile([128, 1], F32)
            nc.vector.memset(expbias[:], EXP_SHIFT)

            # ones columns of v8 (sumexp augmentation); first tiles here,
            # the rest written inside batch-0 v_units
            nc.vector.memset(v8[:, 0, :, :, 0:DK], 1.0)

            # ---- work units (closures) for PE-filler interleaving ----
            def xload_unit(b, half):
                def f():
                    nsl = slice(half * 512, half * 512 + 512)
                    [nc.sync, nc.gpsimd][half].dma_start(
                        x16[:, :, nsl],
                        x16_d[b].rearrange("(c p) n -> p c n", p=128)
                        [:, :, nsl])
                return f

            def xpbload_unit(b, half):
                def f():
                    nsl = slice(half * 512, half * 512 + 512)
                    [nc.sync, nc.gpsimd][half].dma_start(
                        xpb_r[b][:, :, nsl],
                        xpb_d[b].rearrange("(c p) n -> p c n", p=128)
                        [:, :, nsl])
                return f

            def qkT_unit(p, qk, nh):
                def f():
                    nsl = slice(nh * 512, nh * 512 + 512)
                    ps = ps_acc.tile([128, 512], F32, tag="acc", name="qk_ps")
                    for ch in range(NCH):
                        nc.tensor.matmul(
                            ps[:], wqk[:, p, ch, qk, :], x16[:, ch, nsl],
                            start=(ch == 0), stop=(ch == NCH - 1))
                    if qk == 0:
                        nc.vector.tensor_scalar(
                            out=qpair[p][:, nsl], in0=ps[:],
                            scalar1=bqkt[:, 0, p:p + 1], scalar2=None,
                            op0=ALU.add)
                    else:
                        nc.vector.tensor_scalar(
                            out=kpad[p][0][0:64, nsl], in0=ps[0:64, :],
                            scalar1=bqkt[0:64, 1, p:p + 1], scalar2=None,
                            op0=ALU.add)
                        nc.vector.tensor_scalar(
                            out=kpad[p][1][64:128, nsl], in0=ps[64:128, :],
                            scalar1=bqkt[64:128, 1, p:p + 1], scalar2=None,
                            op0=ALU.add)
                return f

            def v_unit(t, first=False):
                def f():
                    if first and t >= 2:
                        nc.vector.memset(v8[:, t // 2, t % 2, :, 0:DK], 1.0)
                    ps = ps_acc.tile([128, 512], F32, tag="acc", name="v_ps")
                    for ch in range(NCH):
                        nc.tensor.matmul(
                            ps[:], x16[:, ch, t * 128:(t + 1) * 128],
                            wv[:, ch, :],
                            start=(ch == 0), stop=(ch == NCH - 1))
                    nc.vector.tensor_copy(
                        v8[:, t // 2, t % 2, :, DK:128],
                        ps[:].rearrange("p (h d) -> p h d", h=N_HEADS))
                return f

            def out_units(b):
                units = []

                def mk(ct, nh):
                    def f():
                        csl = slice(ct * 128, (ct + 1) * 128)
                        nsl = slice(nh * 512, nh * 512 + 512)
                        ps = ps_acc.tile([128, 512], F32, tag="acc",
                                         name="out_ps")
                        for c2 in range(NCH2):
                            nc.tensor.matmul(
                                ps[:], wo[c2][:, :, csl],
                                resc8[b][c2][:, :, nsl],
                                start=(c2 == 0), stop=(c2 == NCH2 - 1),
                                perf_mode=PM.DoubleRow)
                        out_sb = sbwork.tile([128, 512], F16, tag="out",
                                             name="out_sb")
                        nc.vector.tensor_add(out_sb[:], ps[:],
                                             xpb_r[b][:, ct, nsl])
                        [nc.sync, nc.gpsimd][ct % 2].dma_start(
                            y_d[b, csl, nsl], out_sb[:])
                    return f

                for nh in range(2):
                    for ct in range(NCH):
                        units.append(mk(ct, nh))
                return units[:NCH], units[NCH:]

            filler = deque()

            def inject(k=1):
                for _ in range(min(k, len(filler))):
                    filler.popleft()()

            def norm_group(b, p, ic, res_ps):
                isl = slice(ic * 512, ic * 512 + 512)
                for s in range(2):
                    rcp = sbwork.tile([64, 512], F32, tag="rcp", bufs=2,
                                      name="rcp_sb")
                    nc.vector.reciprocal_approx_fast(
                        out=rcp[:], in_=res_ps[s][0:DK, :])
                    nc.vector.tensor_mul(
                        resc8[b][p // 2][s * 64:(s + 1) * 64, p % 2, isl],
                        res_ps[s][DK:128, :], rcp[:])

            # ---- emission schedule: one flat attention stream ----
            wo = []
            wo_units = []
            for c2 in range(NCH2):
                w = const.tile([128, 2, C], F8, name=f"wout{c2}")
                wo.append(w)

                def mk_wo(c2=c2, w=w):
                    def f():
                        nc.sync.dma_start(w[:], wout_d[c2])
                    return f
                wo_units.append(mk_wo())

            def qkts(p):
                return [qkT_unit(p, qk, nh) for qk in range(2)
                        for nh in range(2)]

            with nc.named_scope("b0_proj"):
                # only the nh=0 halves up front; scores t=0..3 need just these
                qkT_unit(0, 0, 0)()
                qkT_unit(0, 1, 0)()

            b0_nh0, b0_nh1 = out_units(0)
            b1_nh0, _ = out_units(1)       # nh=1 handled inline at the tail
            b0_all = b0_nh0 + b0_nh1

            groups = [(b, p, ic) for b in range(NB) for p in range(NPAIR)
                      for ic in range(2)]
            group_fill = {
                0: [qkT_unit(0, 0, 1), qkT_unit(0, 1, 1)] + qkts(1),
                2: qkts(2),
                3: qkts(3),
                4: [xload_unit(1, 0), xload_unit(1, 1)],
                5: wo_units,
                6: qkts(0),
                7: qkts(1) + [xpbload_unit(0, 0)],
                9: qkts(2) + [xpbload_unit(0, 1)],
                10: qkts(3) + [xpbload_unit(1, 0)],
                11: b0_all[0:2] + [xpbload_unit(1, 1)],
                12: b0_all[2:4],
                13: b0_all[4:6], 14: b0_all[6:8],
                15: b1_nh0,
            }
            # v units are heavy (4 fp16 matmuls): spread 1 per step.
            # batch-0: steps 2..9 (v(t) needed by attnv(g0,t//2) at pop t+3).
            # batch-1: v(t) legal after attnv(b0 g7, t2=t//2) reads the v8
            # slot (WAR) and before attnv(b1 g8, t2=t//2) (RAW): pop v0-v3
            # on g7's odd-step pops, v4-v7 on g8 pushes t=0..3.
            vpost_a = deque(v_unit(t) for t in range(NT // 2))
            vpost_b = deque(v_unit(t) for t in range(NT // 2, NT))
            pre = deque(v_unit(t, first=True) for t in range(NT))

            nsteps = len(groups) * NT
            pending = deque()
            res_of = {}
            exp_of = {}
            with nc.named_scope("attn_stream"):
                for k in range(nsteps + 2):
                    g, t = divmod(k, NT)
                    if k < nsteps:
                        b, p, ic = groups[g]
                        if t == 0:
                            if g in group_fill:
                                filler.extend(group_fill[g])
                            res_of[g] = [
                                ps_res.tile([128, 512], F32, tag="res",
                                            name=f"res{g}_{s}")
                                for s in range(2)]
                        isl = slice(ic * 512, ic * 512 + 512)
                        js = slice(t * 128, (t + 1) * 128)
                        s_ps = ps_s.tile([128, N], F32, tag="s", name="s_ps")
                        nc.tensor.matmul(s_ps[:, 0:512], kpad[p][0][:, js],
                                         qpair[p][:, isl],
                                         start=True, stop=True)
                        nc.tensor.matmul(s_ps[:, 512:1024], kpad[p][1][:, js],
                                         qpair[p][:, isl],
                                         start=True, stop=True)
                        if t % 2 == 0:
                            exp_of[(g, t // 2)] = sbexp.tile(
                                [128, 2, 2, 512], F8E5, tag="exp",
                                name="exp_sb")
                        nc.scalar.activation(
                            out=exp_of[(g, t // 2)][:, t % 2, :, :],
                            in_=s_ps[:], func=AF.Exp, bias=expbias[:],
                            scale=SCALE)
                        if pre and (g == 0 and t >= 1 or g == 1 and t == 0):
                            pre.popleft()()
                        if vpost_b and g == 8 and t <= 3:
                            vpost_b.popleft()()
                        pending.append((g, t))
                    if len(pending) == 3 or (k >= nsteps and pending):
                        pg, pt = pending.popleft()
                        pb, pp, pic = groups[pg]
                        if pt % 2 == 1:
                            t2 = pt // 2
                            e = exp_of.pop((pg, t2))
                            for s in range(2):
                                nc.tensor.matmul(
                                    res_of[pg][s][:],
                                    v8[:, t2, :, 2 * pp + s, :],
                                    e[:, :, s, :],
                                    start=(t2 == 0), stop=(t2 == NT2 - 1),
                                    perf_mode=PM.DoubleRow)
                            if pg == 7 and vpost_a:
                                vpost_a.popleft()()
                        if pt == NT - 1:
                            norm_group(pb, pp, pic, res_of.pop(pg))
                        if pg not in (7, 8) and 2 <= t <= 6:
                            inject(1)

            with nc.named_scope("b1_out"):
                inject(len(filler))
                # batch-1 second-half out projection, latency-trimmed:
                # accumulators live in the freed score banks; the chunk-pair-0
                # DR and pair-2 matmuls pre-run; only pair-3 waits on the
                # final norm.  Residual adds batched 2 chunks at a time;
                # stores ride sync + the now-idle scalar queue so gpsimd's
                # end-of-program DMA drain overlaps the tail compute.
                nsl = slice(512, 1024)
                tp = [ps_s.tile([128, N], F32, tag="s", name=f"tailp{h}")
                      for h in range(2)]

                def acc_of(ct):
                    return tp[ct // 2][:, (ct % 2) * 512:(ct % 2) * 512 + 512]

                for ct in range(NCH):
                    csl = slice(ct * 128, (ct + 1) * 128)
                    nc.tensor.matmul(acc_of(ct), wo[0][:, :, csl],
                                     resc8[1][0][:, :, nsl],
                                     start=True, stop=False,
                                     perf_mode=PM.DoubleRow)
                    nc.tensor.matmul(acc_of(ct), wo[1][:, 0, csl],
                                     resc8[1][1][:, 0, nsl],
                                     start=False, stop=False)
                for ct in range(NCH):
                    csl = slice(ct * 128, (ct + 1) * 128)
                    nc.tensor.matmul(acc_of(ct), wo[1][:, 1, csl],
                                     resc8[1][1][:, 1, nsl],
                                     start=False, stop=True)
                for h in range(2):
                    out_sb = sbwork.tile([128, 2, 512], F16, tag="out",
                                         name="out_sb2")
                    nc.vector.tensor_add(
                        out_sb[:],
                        tp[h][:].rearrange("p (a n) -> p a n", a=2),
                        xpb_r[1][:, 2 * h:2 * h + 2, nsl])
                    for r in range(2):
                        ct = 2 * h + r
                        csl = slice(ct * 128, (ct + 1) * 128)
                        [nc.sync, nc.scalar][r].dma_start(
                            y_d[1, csl, nsl], out_sb[:, r, :])

    nc.finalize()
    return nc


_NC = None


def _get_nc():
    global _NC
    if _NC is None:
        _NC = build()
    return _NC


def make_in_maps(x, W_qkv, b_qkv, W_out, b_out):
    x = np.ascontiguousarray(np.asarray(x, np.float32)).reshape(16, C, N)
    b_out = np.asarray(b_out, np.float32)
    w3 = np.asarray(W_qkv, np.float32).reshape(C, N_HEADS, 3, DK)
    # w_qk[p, pair, ch, qk, 64u+d] = w3[ch*128+p, 2*pair+u, qk, d]
    wqk = np.stack([w3[:, :, 0], w3[:, :, 1]], axis=1)  # (C, 2(qk), H, DK)
    wqk = wqk.reshape(C, 2, NPAIR, 2 * DK)              # merge (u, d) -> m
    wqk = wqk.reshape(NCH, 128, 2, NPAIR, 2 * DK)       # ch, p, qk, pr, m
    w_qk = np.ascontiguousarray(wqk.transpose(1, 3, 0, 2, 4))
    # w_v[p, ch, 64h+d] = w3[ch*128+p, h, 2, d]
    wv = w3[:, :, 2].reshape(NCH, 128, C)
    w_v = np.ascontiguousarray(wv.transpose(1, 0, 2))
    b3 = np.asarray(b_qkv, np.float32).reshape(N_HEADS, 3, DK)
    b_qk_t = np.ascontiguousarray(
        np.stack([b3[:, 0], b3[:, 1]], axis=0)
        .reshape(2, NPAIR, 128).transpose(2, 0, 1))
    # b_v is never added to v on device: softmax rows sum to 1, so
    # attn@(v + b_v) = attn@v + b_v, and b_v@W_out folds into the residual.
    b_v = b3[:, 2].reshape(C)
    W_out = np.asarray(W_out, np.float32)
    # w_out[c2, p, r, c] = W_out[(2*c2+r)*128+p, c]
    w_o = np.ascontiguousarray(
        W_out.reshape(NCH2, 2, 128, C).transpose(0, 2, 1, 3))
    bfull = b_out + b_v @ W_out
    xpb = np.ascontiguousarray(x + bfull[None, :, None]).astype(np.float16)
    x16 = x.astype(np.float16)
    maps = []
    for core in range(NCORES):
        sl = slice(core * NB, (core + 1) * NB)
        maps.append({
            "x16": x16[sl],
            "xpb": xpb[sl],
            "w_qk": w_qk.astype(np.float16),
            "b_qk_t": b_qk_t,
            "w_v": w_v.astype(np.float16),
            "w_out": w_o.astype(ml_dtypes.float8_e4m3),
            "zeros": np.zeros((1, N), np.float16),
        })
    return maps


def run_on_hw(in_maps, **kwargs):
    nc = _get_nc()
    return bass_utils.run_bass_kernel_spmd(
        nc, in_maps, core_ids=list(range(NCORES)), **kwargs)


def kernel(x, W_qkv, b_qkv, W_out, b_out):
    res = run_on_hw(make_in_maps(x, W_qkv, b_qkv, W_out, b_out))
    y = np.concatenate([r["y"] for r in res.results], axis=0)  # (16, C, N)
    return y.reshape(16, C, 32, 32).astype(np.float32)
